# revision 2
# baseline (speedup 1.0000x reference)
"""Cepstrum -> minimum-phase impulse response, v2.

Same spectral math as v1 (h = IDFT_512(exp(DFT_512(c)))), restructured to
unload the ACT and DVE engines:

  - Host fold: device ships E_n (n=1..256) and O_n (n=1..256; O_256=0) per
    row; host computes h_n = E+O, h_{512-n} = E-O, h_0 = exp(c_0) exactly.
    The v1 ACT staging copies and DVE add/subs disappear.
  - E and O pack into ONE fp32 PSUM bank per row-chunk ([128, 512]: E at
    cols 0:256, O at 256:512), extracted by a single [128,512] copy
    (3 row-chunks on DVE, 1 on ACT to balance the engines).
  - R and I accumulate into [128, 1024] fp32 PSUM tiles (bc0|bc1 side by
    side, 2 banks each), so exp/sin/cos are 3 ACT ops of [128,1024] per
    supertile instead of 6 of [128,512].
  - One output DMA per supertile ([128, 2048]) instead of 4.

Device pipeline per 512-row supertile:
  - fwd:  R/I[bins, rows] = Wcos/Wsin-stationary matmuls over cT (fp16 in)
  - ACT:  A = exp(R), Sn = sin2pi(I/2pi), Cs = sin2pi(I/2pi + 1/4)
  - DVE:  HRe = A*Cs, HIm = A*Sn  (fp16 2x)
  - inv:  per rc: eo[128,512] f32 psum: E chain -> [:, 0:256],
          O chain -> [:, 256:512]; one copy to SBUF; one DMA per supertile.
"""

import os
import sys
from contextlib import ExitStack

import numpy as np

for _p in ("/opt/trn_rl_repo", "/root/.axon_site/_ro/trn_rl_repo"):
    if os.path.isdir(_p) and _p not in sys.path:
        sys.path.insert(0, _p)

from concourse import bacc, mybir, tile  # noqa: E402
from concourse.bass_utils import run_bass_kernel_spmd  # noqa: E402

B_TOTAL = 131072
N_CORES = 8
B_CORE = B_TOTAL // N_CORES  # 16384
M1 = 256          # cepstral coefficients per row (M+1)
N_OUT = 512       # impulse response length
L = 512           # DFT length
NB = L // 2       # matmul-handled bins 1..NB (bin 0 on host)
NEO = 256         # E ships n=1..256; O ships n=1..256 (O_256 = 0)
ST_ROWS = 512     # rows per supertile
N_ST = B_CORE // ST_ROWS  # 32

F32 = mybir.dt.float32
F16 = mybir.dt.float16

_cache: dict = {}

TWO_PI = 2.0 * np.pi


def _install_sin2pi_patches():
    """Keep all activations in ONE ACT table set (exp_and_friends = {exp,
    sin2pi}); rewrite Sin -> Sin2pi in the BIR (args already scaled)."""
    if _cache.get("patched"):
        return
    import concourse.bacc as _bacc
    import concourse.bass2jax as _b2j

    SIN = mybir.ActivationFunctionType.Sin
    EXP = mybir.ActivationFunctionType.Exp
    _orig_tables = _bacc.get_activation_tables

    def tables_patched(arch):
        t = {k: set(v) for k, v in _orig_tables(arch).items()}
        for k in t:
            t[k].discard(SIN)
            if k != "exp_and_friends":
                t[k].discard(EXP)
        if "exp_and_friends" in t:
            t["exp_and_friends"] |= {SIN, EXP}
        return t

    _bacc.get_activation_tables = tables_patched

    _orig_compile = _b2j.compile_bir_kernel

    def compile_patched(bir_json, *a, **kw):
        if isinstance(bir_json, bytes):
            if b'"wcf"' in bir_json:
                bir_json = bir_json.replace(b'"func":"Sin"', b'"func":"Sin2pi"')
        elif '"wcf"' in bir_json:
            bir_json = bir_json.replace('"func":"Sin"', '"func":"Sin2pi"')
        return _orig_compile(bir_json, *a, **kw)

    _b2j.compile_bir_kernel = compile_patched
    _cache["patched"] = True


def _host_weights():
    if "w" in _cache:
        return _cache["w"]
    d = np.arange(M1, dtype=np.float64)
    k = np.arange(1, NB + 1, dtype=np.float64)
    th = 2.0 * np.pi * np.outer(d, k) / L           # (256, NB)
    wc = np.cos(th)
    ws = -np.sin(th)
    n = np.arange(1, NEO + 1, dtype=np.float64)     # n = 1..256
    thi = 2.0 * np.pi * np.outer(k, n) / L          # (NB, NEO)
    w = np.where(k == NB, 1.0, 2.0)[:, None] / L
    ci = w * np.cos(thi)
    si = -w * np.sin(thi)
    si[:, NEO - 1] = 0.0                            # O_256 = 0 (pad col)
    ci16 = ci.astype(np.float16)
    si16 = si.astype(np.float16)
    # Coherent fp16-rounding error of ci against the H ~= 1 background.
    corrE = -(ci16.astype(np.float64) - ci).sum(0)  # (256,) for n=1..256
    _cache["w"] = (
        wc.astype(np.float16),
        ws.astype(np.float16),
        ci16,
        si16,
        corrE.astype(np.float32),
    )
    return _cache["w"]


def _build(n_st=N_ST, repeat=1, mul_cs_first=False, inv_e_first=True, him_gpsimd=False):
    _install_sin2pi_patches()
    nc = bacc.Bacc(
        "TRN2", target_bir_lowering=False, debug=False, num_devices=N_CORES
    )
    ct_ap = nc.dram_tensor("ct", [M1, n_st * ST_ROWS], F16, kind="ExternalInput").ap()
    wc_ap = nc.dram_tensor("wcf", [M1, NB], F16, kind="ExternalInput").ap()
    ws_ap = nc.dram_tensor("wsf", [M1, NB], F16, kind="ExternalInput").ap()
    ci_ap = nc.dram_tensor("cif", [NB, NEO], F16, kind="ExternalInput").ap()
    si_ap = nc.dram_tensor("sif", [NB, NEO], F16, kind="ExternalInput").ap()
    # out row st*128+p holds [rc0 | rc1 | rc2 | rc3] blocks of width 512:
    # block cols 0:256 = E_n (n=1..256), 256:512 = O_n (n=1..256), for
    # batch row st*512 + rc*128 + p.
    h_ap = nc.dram_tensor(
        "h", [n_st * 128, 4 * 512], F16, kind="ExternalOutput"
    ).ap()

    EXP = mybir.ActivationFunctionType.Exp
    SIN = mybir.ActivationFunctionType.Sin

    with tile.TileContext(nc) as tc, ExitStack() as ctx:
        const = ctx.enter_context(tc.tile_pool(name="const", bufs=1))
        ctp = ctx.enter_context(tc.tile_pool(name="ctp", bufs=4))
        actp = ctx.enter_context(tc.tile_pool(name="actp", bufs=4))
        hp = ctx.enter_context(tc.tile_pool(name="hp", bufs=3))
        outp = ctx.enter_context(tc.tile_pool(name="outp", bufs=4))
        ps_r = ctx.enter_context(tc.tile_pool(name="ps_r", bufs=1, space="PSUM"))
        ps_i = ctx.enter_context(tc.tile_pool(name="ps_i", bufs=2, space="PSUM"))
        ps_eo = ctx.enter_context(tc.tile_pool(name="ps_eo", bufs=2, space="PSUM"))

        # constants
        wc_sb = [const.tile([128, NB], F16, tag=f"wc{d}", name=f"wc{d}") for d in range(2)]
        ws_sb = [const.tile([128, NB], F16, tag=f"ws{d}", name=f"ws{d}") for d in range(2)]
        for d in range(2):
            nc.sync.dma_start(wc_sb[d][:], wc_ap[d * 128:(d + 1) * 128, :])
            nc.sync.dma_start(ws_sb[d][:], ws_ap[d * 128:(d + 1) * 128, :])
        ci_sb = [const.tile([128, NEO], F16, tag=f"ci{b}", name=f"ci{b}") for b in range(2)]
        si_sb = [const.tile([128, NEO], F16, tag=f"si{b}", name=f"si{b}") for b in range(2)]
        for b in range(2):
            nc.sync.dma_start(ci_sb[b][:], ci_ap[b * 128:(b + 1) * 128, :])
            nc.sync.dma_start(si_sb[b][:], si_ap[b * 128:(b + 1) * 128, :])
        zb = const.tile([128, 1], F32, tag="zb")
        nc.gpsimd.memset(zb[:], 0.0)
        quarter = const.tile([128, 1], F32, tag="quarter")
        nc.gpsimd.memset(quarter[:], 0.25)

        # 2-deep software pipeline: iteration j emits
        #   fwd(j): cT DMA, R/I matmuls, ACT trio (exp/sin/cs)
        #   inv-MMs(j-2) + DVE muls(j-1) + extraction copies(j-2) + DMA(j-2)
        # which stretches the serial chain cs->muls->inv->R->exp over three
        # cycles, leaving engine busy-time as the only bound. DVE FIFO per
        # iter: [muls(j-1), copies(j-2)] so muls never queue behind copies.
        fwd_st = None  # (a_sb, sn_sb, cs_sb, row) ACT done, muls not yet
        inv_st = None  # (hre, him, row)           muls done, inv not yet
        n_iter = n_st * repeat
        for sti in range(n_iter + 2):
            st = sti % n_st
            r0 = st * ST_ROWS
            new_fwd = None
            if sti < n_iter:
                cT16 = []
                for d in range(2):
                    t = ctp.tile([128, ST_ROWS], F16, tag=f"cT{d}", name="cT16")
                    nc.sync.dma_start(
                        t[:], ct_ap[d * 128:(d + 1) * 128, r0:r0 + ST_ROWS]
                    )
                    cT16.append(t)
                r_ps = ps_r.tile([128, 1024], F32, tag="R")
                i_ps = ps_i.tile([128, 1024], F32, tag="I")
                for bc in range(2):
                    sl = slice(bc * 128, (bc + 1) * 128)
                    for d in range(2):
                        nc.tensor.matmul(
                            r_ps[:, bc * 512:(bc + 1) * 512],
                            wc_sb[d][:, sl], cT16[d][:],
                            start=(d == 0), stop=(d == 1),
                        )
                for bc in range(2):
                    sl = slice(bc * 128, (bc + 1) * 128)
                    for d in range(2):
                        nc.tensor.matmul(
                            i_ps[:, bc * 512:(bc + 1) * 512],
                            ws_sb[d][:, sl], cT16[d][:],
                            start=(d == 0), stop=(d == 1),
                        )
                a_sb = actp.tile([128, 1024], F16, tag="A")
                sn_sb = actp.tile([128, 1024], F16, tag="Sn")
                cs_sb = actp.tile([128, 1024], F16, tag="Cs")
                nc.scalar.activation(a_sb[:], r_ps[:], EXP, bias=zb[:])
                nc.scalar.activation(sn_sb[:], i_ps[:], SIN, bias=zb[:],
                                     scale=float(1.0 / TWO_PI))
                nc.scalar.activation(cs_sb[:], i_ps[:], SIN, bias=quarter[:],
                                     scale=float(1.0 / TWO_PI))
                new_fwd = (a_sb, sn_sb, cs_sb, st * 128)

            if inv_st is not None:
                hre_p, him_p, row_p = inv_st
                o_big = outp.tile([128, 4 * 512], F16, tag="o_big")
                eos = []
                for rc in range(4):
                    eo = ps_eo.tile([128, 512], F32, tag="eo")
                    chains = [(slice(0, NEO), hre_p, ci_sb),
                              (slice(NEO, 512), him_p, si_sb)]
                    if not inv_e_first:
                        chains.reverse()
                    for osl, hsrc, wsb in chains:
                        for bc in range(2):
                            hsl = slice(bc * 512 + rc * 128,
                                        bc * 512 + (rc + 1) * 128)
                            nc.tensor.matmul(
                                eo[:, osl], hsrc[:, hsl], wsb[bc][:],
                                start=(bc == 0), stop=(bc == 1),
                            )
                    eos.append(eo)

            new_inv = None
            if fwd_st is not None:
                a_p, sn_p, cs_p, frow = fwd_st
                hre = hp.tile([128, 1024], F16, tag="HRe")
                him = hp.tile([128, 1024], F16, tag="HIm")
                if him_gpsimd:
                    nc.gpsimd.tensor_mul(him[:], a_p[:], sn_p[:])
                    nc.vector.tensor_mul(hre[:], a_p[:], cs_p[:])
                elif mul_cs_first:
                    nc.vector.tensor_mul(hre[:], a_p[:], cs_p[:])
                    nc.vector.tensor_mul(him[:], a_p[:], sn_p[:])
                else:
                    nc.vector.tensor_mul(him[:], a_p[:], sn_p[:])
                    nc.vector.tensor_mul(hre[:], a_p[:], cs_p[:])
                new_inv = (hre, him, frow)

            if inv_st is not None:
                for rc in range(4):
                    nc.vector.tensor_copy(
                        o_big[:, rc * 512:(rc + 1) * 512], eos[rc][:]
                    )
                nc.sync.dma_start(h_ap[row_p:row_p + 128, :], o_big[:])

            inv_st = new_inv
            fwd_st = new_fwd

    nc.compile()
    return nc


def _get_nc(n_st=N_ST):
    key = ("nc", n_st)
    if key not in _cache:
        _cache[key] = _build(n_st)
    return _cache[key]


def _in_maps(c):
    wc, ws, ci, si, _ = _host_weights()
    maps = []
    for i in range(N_CORES):
        ct = np.ascontiguousarray(
            c[i * B_CORE:(i + 1) * B_CORE].astype(np.float16).T
        )
        maps.append({"ct": ct, "wcf": wc, "wsf": ws, "cif": ci, "sif": si})
    return maps


def _assemble(hdev, c):
    """Host fold: h_n = E+O (n=1..255), h_256 = E_256, h_{512-n} = E-O;
    h_0 = exp(c_0) exactly; add DC bin + corrE."""
    _, _, _, _, corrE = _host_weights()
    B = c.shape[0]
    n_st = N_ST
    d = hdev.reshape(N_CORES, n_st, 128, 4, 512)
    d = d.transpose(0, 1, 3, 2, 4).reshape(B, 512)
    E = d[:, 0:NEO].astype(np.float32)        # n = 1..256
    O = d[:, NEO:512].astype(np.float32)      # n = 1..256 (col 255 ~ 0)
    c64 = c.astype(np.float64)
    dc = (np.exp(c64.sum(axis=1)) / L).astype(np.float32)
    add = dc[:, None] + corrE[None, :]        # (B, 256)
    out = np.empty((B, N_OUT), np.float32)
    out[:, 0] = np.exp(c64[:, 0]).astype(np.float32)
    out[:, 1:257] = E + O + add
    # h[512-n] = E_n - O_n + add_n for n = 1..255 -> cols 257..511
    out[:, 257:] = (E - O + add)[:, 254::-1]
    return out


def kernel(c):
    c = np.ascontiguousarray(np.asarray(c), dtype=np.float32)
    assert c.shape == (B_TOTAL, M1), c.shape
    nc = _get_nc()
    res = run_bass_kernel_spmd(nc, _in_maps(c), list(range(N_CORES)))
    hdev = np.concatenate(
        [res.results[i]["h"] for i in range(N_CORES)], axis=0
    )
    return _assemble(hdev, c)
